# revision 1
# baseline (speedup 1.0000x reference)
"""Trainium2 Bass kernel for nn_CrossModalAttention (B=16384, GNN=512, TR=768, F=1024).

Math (seq_len==1 degenerate attention, see reference):
    gp = g @ Wg.T + bg                       [B, F]
    tp = t @ Wt.T + bt                       [B, F]
    ga = (tp @ Wv.T + bv) @ Wo.T + bo        (attention(g, t, t))
    ta = (gp @ Wv.T + bv) @ Wo.T + bo
    h  = gelu([ga, ta] @ W1.T + b1)
    out = h @ W2.T + b2 + gp + tp

The attention block is affine, so it folds into W1 on the host:
    M1 = W1[:, :F] @ Wo @ Wv   (multiplies tp)
    M2 = W1[:, F:] @ Wo @ Wv   (multiplies gp)
    c  = (W1[:, :F] + W1[:, F:]) @ (Wo @ bv + bo) + b1
    h  = gelu(M1 @ tp.T + M2 @ gp.T + c)     (transposed layout)

Device kernel works in transposed layout [feature, batch] so the matmul
contraction dim always lands on SBUF partitions; host transposes in/out.
Data parallel over 8 cores: each core owns 2048 batch rows.
"""

import sys

import numpy as np

for _p in ("/opt/trn_rl_repo", "/root/.axon_site/_ro/trn_rl_repo"):
    if _p not in sys.path:
        sys.path.append(_p)

import ml_dtypes

import concourse.bass as bass
import concourse.mybir as mybir
import concourse.tile as tile
from concourse.bass import ts
from concourse.bass_utils import run_bass_kernel_spmd

B = 16384
GNN = 512
TR = 768
F = 1024
N_CORES = 8
B_LOC = B // N_CORES  # 2048
P = 128

# Stage dtypes: AB = the gp/tp projections (dominant output terms),
# CD = the folded-attention/fusion branch (small contribution to output).
# "bf16x2" = hi/lo bf16 split of inputs+weights, 3 matmuls per K-tile
# (drops only the lo*lo term): ~1e-5 rel err at 3x bf16 cost.
AB_DT = "f32r"  # "f32r" | "bf16" | "f32" | "bf16x2"
CD_DT = "bf16"  # "bf16" | "f32r" | "f32"
NB = 512  # batch-column block per step
PSUM_BUFS = 8
IO_BUFS = 1
AF = mybir.ActivationFunctionType


def _np_dt(sdt):
    return ml_dtypes.bfloat16 if sdt == "bf16" else np.float32


def _mb_dt(sdt):
    return {
        "bf16": mybir.dt.bfloat16,
        "f32r": mybir.dt.float32r,
        "f32": mybir.dt.float32,
    }[sdt]


def _mm_cast(ap, sdt):
    """Bitcast a float32 AP to float32r for reduced-precision full-rate matmul."""
    if sdt == "f32r":
        return ap.bitcast(mybir.dt.float32r)
    return ap


_DMA_OPCODES = ("DMACopy", "DMATranspose", "EventSemaphore", "TriggeredCopy")


def _legalize_waits(bir: dict) -> dict:
    """Walrus on this stack accepts only ONE sync-wait per engine instruction
    ("Too many sync wait commands"). Hoist extra waits onto standalone
    EventSemaphore ops (what nc.<engine>.wait_ge emits) on the same engine."""
    ctr = 0

    def hoist(out, inst, w):
        nonlocal ctr
        ctr += 1
        out.append(
            {
                "debug": inst.get("debug", 0),
                "engine": inst["engine"],
                "ins": [],
                "outs": [],
                "name": f"I-lgw-{ctr}",
                "opcode": "EventSemaphore",
                "sync_info": {"on_update": [], "on_wait": [w]},
            }
        )

    for fn in bir["functions"]:
        for blk in fn["blocks"]:
            out = []
            for inst in blk["instructions"]:
                si = inst.get("sync_info")
                waits = (si.get("on_wait") or []) if si else []
                op = inst.get("opcode")
                if op == "EventSemaphore":
                    pass
                elif op in ("DMACopy", "DMATranspose", "TriggeredCopy"):
                    # keep one wait (prefer a queue DMA* sem) on the descriptor,
                    # hoist the rest onto the issuing sequencer
                    if len(waits) > 1:
                        keep = [w for w in waits if w["ant_name"].startswith("DMA")]
                        drop = [w for w in waits if not w["ant_name"].startswith("DMA")]
                        if not keep:
                            keep = [waits[-1]]
                            drop = waits[:-1]
                        while len(keep) > 1:
                            drop.append(keep.pop(0))
                        for w in drop:
                            hoist(out, inst, w)
                        si["on_wait"] = keep
                elif len(waits) > 1:
                    for w in waits[:-1]:
                        hoist(out, inst, w)
                    si["on_wait"] = waits[-1:]
                out.append(inst)
            blk["instructions"] = out
    return bir


def _attach_wait_legalizer(nc):
    import json as _json

    orig_fn = nc.to_json_bytes

    def _patched():
        bir = _json.loads(orig_fn())
        _legalize_waits(bir)
        return _json.dumps(bir).encode()

    nc.to_json_bytes = _patched


def build_module(repeat=1):
    nc = bass.Bass()
    f32 = mybir.dt.float32
    # tensors consumed by an fp32r matmul must themselves be declared fp32r
    # end-to-end (walrus birverifier "not rounded to FP32r" check)
    ab_io = _mb_dt(AB_DT)
    cd_io = _mb_dt(CD_DT)

    gT = nc.dram_tensor("gT", [GNN, B_LOC], ab_io, kind="ExternalInput")
    tT = nc.dram_tensor("tT", [TR, B_LOC], ab_io, kind="ExternalInput")
    wgT = nc.dram_tensor("wgT", [GNN, F], ab_io, kind="ExternalInput")
    wtT = nc.dram_tensor("wtT", [TR, F], ab_io, kind="ExternalInput")
    mcT = nc.dram_tensor("mcT", [2 * F, F], cd_io, kind="ExternalInput")
    w2T = nc.dram_tensor("w2T", [F, F], cd_io, kind="ExternalInput")
    bg = nc.dram_tensor("bg", [F], f32, kind="ExternalInput")
    bt = nc.dram_tensor("bt", [F], f32, kind="ExternalInput")
    cv = nc.dram_tensor("cv", [F], f32, kind="ExternalInput")
    b2 = nc.dram_tensor("b2", [F], f32, kind="ExternalInput")
    outT = nc.dram_tensor("outT", [F, B_LOC], f32, kind="ExternalOutput")

    KG = GNN // P  # 4
    KT = TR // P  # 6
    KF = F // P  # 8
    NBLK = B_LOC // NB

    g_ap = gT[:].rearrange("(k p) b -> p k b", p=P)
    t_ap = tT[:].rearrange("(k p) b -> p k b", p=P)
    out_ap = outT[:].rearrange("(k p) b -> p k b", p=P)

    with tile.TileContext(nc) as tc:
        with (
            tc.tile_pool(name="const", bufs=1) as const,
            tc.tile_pool(name="io", bufs=IO_BUFS) as io,
            tc.tile_pool(name="act", bufs=1) as act,
            tc.tile_pool(name="psum", bufs=PSUM_BUFS, space="PSUM") as psum,
        ):
            wg = const.tile([P, KG, F], _mb_dt(AB_DT))
            nc.sync.dma_start(out=wg, in_=wgT[:].rearrange("(k p) f -> p k f", p=P))
            wt = const.tile([P, KT, F], _mb_dt(AB_DT))
            nc.sync.dma_start(out=wt, in_=wtT[:].rearrange("(k p) f -> p k f", p=P))
            bg_t = const.tile([P, KF], f32)
            nc.sync.dma_start(out=bg_t, in_=bg[:].rearrange("(k p) -> p k", p=P))
            bt_t = const.tile([P, KF], f32)
            nc.sync.dma_start(out=bt_t, in_=bt[:].rearrange("(k p) -> p k", p=P))
            cv_t = const.tile([P, KF], f32)
            nc.sync.dma_start(out=cv_t, in_=cv[:].rearrange("(k p) -> p k", p=P))
            b2_t = const.tile([P, KF], f32)
            nc.sync.dma_start(out=b2_t, in_=b2[:].rearrange("(k p) -> p k", p=P))
            mc = const.tile([P, 2 * KF, F], _mb_dt(CD_DT))
            nc.sync.dma_start(out=mc, in_=mcT[:].rearrange("(k p) f -> p k f", p=P))
            w2 = const.tile([P, KF, F], _mb_dt(CD_DT))
            nc.sync.dma_start(out=w2, in_=w2T[:].rearrange("(k p) f -> p k f", p=P))

            for blk in [b for _ in range(repeat) for b in range(NBLK)]:
                bs = slice(blk * NB, (blk + 1) * NB)
                g_in = io.tile([P, KG, NB], wg.dtype, tag="g_in")
                nc.sync.dma_start(out=g_in, in_=g_ap[:, :, bs])
                t_in = io.tile([P, KT, NB], wt.dtype, tag="t_in")
                nc.sync.dma_start(out=t_in, in_=t_ap[:, :, bs])

                act_dt = mybir.dt.float32r if CD_DT == "f32r" else f32
                gp = act.tile([P, KF, NB], act_dt, tag="gp")
                tp = act.tile([P, KF, NB], act_dt, tag="tp")
                if CD_DT == "bf16":
                    gpb = act.tile([P, KF, NB], mybir.dt.bfloat16, tag="gpb")
                    tpb = act.tile([P, KF, NB], mybir.dt.bfloat16, tag="tpb")

                # A: gp = Wg @ g (+bg);  B: tp = Wt @ t (+bt)
                for w_t, x_in, y, yb, b_t, kk in (
                    (wg, g_in, gp, "gpb", bg_t, KG),
                    (wt, t_in, tp, "tpb", bt_t, KT),
                ):
                    for j in range(KF):
                        ps = psum.tile([P, NB], f32, tag="ps")
                        for k in range(kk):
                            nc.tensor.matmul(
                                ps,
                                _mm_cast(w_t[:, k, ts(j, P)], AB_DT),
                                _mm_cast(x_in[:, k, :], AB_DT),
                                start=(k == 0),
                                stop=(k == kk - 1),
                            )
                        nc.scalar.activation(y[:, j, :], ps, AF.Identity, bias=b_t[:, j : j + 1])
                        if CD_DT == "bf16":
                            dst = gpb if yb == "gpb" else tpb
                            nc.vector.tensor_copy(dst[:, j, :], y[:, j, :])

                # C: h = gelu(M2 @ gp + M1 @ tp + c)   (gp half first: ready earlier)
                rhs_g = gpb if CD_DT == "bf16" else gp
                rhs_t = tpb if CD_DT == "bf16" else tp
                h = act.tile([P, KF, NB], mc.dtype, tag="h")
                for j in range(KF):
                    ps = psum.tile([P, NB], f32, tag="ps")
                    for k in range(KF):
                        nc.tensor.matmul(
                            ps,
                            _mm_cast(mc[:, KF + k, ts(j, P)], CD_DT),
                            _mm_cast(rhs_g[:, k, :], CD_DT),
                            start=(k == 0),
                            stop=False,
                        )
                    for k in range(KF):
                        nc.tensor.matmul(
                            ps,
                            _mm_cast(mc[:, k, ts(j, P)], CD_DT),
                            _mm_cast(rhs_t[:, k, :], CD_DT),
                            start=False,
                            stop=(k == KF - 1),
                        )
                    nc.scalar.activation(h[:, j, :], ps, AF.Gelu, bias=cv_t[:, j : j + 1])

                # D: out = W2 @ h + b2 + gp + tp
                # epilogue all on DVE so the out DMA has a single-engine dep
                out_t = io.tile([P, KF, NB], f32, tag="out_t")
                for j in range(KF):
                    ps = psum.tile([P, NB], f32, tag="ps")
                    for k in range(KF):
                        nc.tensor.matmul(
                            ps,
                            _mm_cast(w2[:, k, ts(j, P)], CD_DT),
                            _mm_cast(h[:, k, :], CD_DT),
                            start=(k == 0),
                            stop=(k == KF - 1),
                        )
                    nc.vector.tensor_scalar_add(out_t[:, j, :], ps, b2_t[:, j : j + 1])
                    nc.vector.tensor_add(out_t[:, j, :], out_t[:, j, :], gp[:, j, :])
                    nc.vector.tensor_add(out_t[:, j, :], out_t[:, j, :], tp[:, j, :])
                nc.sync.dma_start(out=out_ap[:, :, bs], in_=out_t)

    _attach_wait_legalizer(nc)
    return nc


def prepare_inputs(gnn_features, transformer_features, Wg, bg, Wt, bt, Wv, bv, Wo, bo, W1, b1, W2, b2):
    """Host-side: fold the affine attention block into W1, transpose everything."""
    f64 = np.float64
    A = Wo.astype(f64) @ Wv.astype(f64)
    W1a = W1[:, :F].astype(f64)
    W1b = W1[:, F:].astype(f64)
    M1 = W1a @ A
    M2 = W1b @ A
    c = (W1a + W1b) @ (Wo.astype(f64) @ bv.astype(f64) + bo.astype(f64)) + b1.astype(f64)

    ab_np = _np_dt(AB_DT)
    cd_np = _np_dt(CD_DT)
    wgT = np.ascontiguousarray(Wg.T).astype(ab_np)
    wtT = np.ascontiguousarray(Wt.T).astype(ab_np)
    mcT = np.ascontiguousarray(np.concatenate([M1.T, M2.T], axis=0).astype(np.float32)).astype(cd_np)
    w2T = np.ascontiguousarray(W2.T).astype(cd_np)

    shared = {
        "wgT": wgT,
        "wtT": wtT,
        "mcT": mcT,
        "w2T": w2T,
        "bg": np.asarray(bg, np.float32),
        "bt": np.asarray(bt, np.float32),
        "cv": c.astype(np.float32),
        "b2": np.asarray(b2, np.float32),
    }
    in_maps = []
    for i in range(N_CORES):
        rows = slice(i * B_LOC, (i + 1) * B_LOC)
        in_maps.append(
            {
                "gT": np.ascontiguousarray(gnn_features[rows].T).astype(ab_np),
                "tT": np.ascontiguousarray(transformer_features[rows].T).astype(ab_np),
                **shared,
            }
        )
    return in_maps


def run(inputs, trace=False, **kw):
    nc = build_module()
    in_maps = prepare_inputs(**inputs)
    res = run_bass_kernel_spmd(nc, in_maps, core_ids=list(range(N_CORES)), trace=trace, **kw)
    out = np.concatenate([r["outT"].T for r in res.results], axis=0).astype(np.float32)
    return out, res


def kernel(**inputs) -> np.ndarray:
    out, _ = run(inputs, trace=False)
    return out



# revision 12
# speedup vs baseline: 3.0841x; 3.0841x over previous
"""Trainium2 Bass kernel for nn_CrossModalAttention (B=16384, GNN=512, TR=768, F=1024).

Math (seq_len==1 degenerate attention, see reference):
    gp = g @ Wg.T + bg                       [B, F]
    tp = t @ Wt.T + bt                       [B, F]
    ga = (tp @ Wv.T + bv) @ Wo.T + bo
    ta = (gp @ Wv.T + bv) @ Wo.T + bo
    h  = gelu([ga, ta] @ W1.T + b1)
    out = h @ W2.T + b2 + gp + tp

The whole attention+first-fusion block is affine in (g, t), so it folds on
the host into a single weight acting on the raw concatenated input x=[g;t]:
    u   = N @ x + c,   N = [M2 @ Wg | M1 @ Wt]  (M1 = W1a @ Wo @ Wv, ...)
    h   = gelu(u)
    S   = Wc @ x + bs, Wc = [Wg | Wt], bs = bg + bt   (= gp + tp)
    out = W2 @ h + b2 + S

Device kernel works transposed [feature, batch]; contraction on partitions.
All matmuls run fp8(e4m3) with perf_mode=DoubleRow (2 contraction tiles per
instruction at half cycles/row). The dominant S term uses an error-
compensated hi/lo fp8 split of both weights and activations:
    S*32 = Wh@xh + Wh@xl + Wl@xh      (drops only the lo*lo term, ~1e-3 rel)
H and D branches run single fp8 (small contribution to the output).
Data parallel over 8 cores: each core owns 2048 batch rows.
"""

import sys

import numpy as np

for _p in ("/opt/trn_rl_repo", "/root/.axon_site/_ro/trn_rl_repo"):
    if _p not in sys.path:
        sys.path.append(_p)

import ml_dtypes

import concourse.bass as bass
import concourse.mybir as mybir
import concourse.tile as tile
from concourse.bass import ts
from concourse.bass_utils import run_bass_kernel_spmd

B = 16384
GNN = 512
TR = 768
F = 1024
N_CORES = 8
B_LOC = B // N_CORES  # 2048
P = 128
X = GNN + TR  # 1280 combined input features
KX = X // P  # 10 contraction tiles for S/H
KF = F // P  # 8
NB = 512
NBLK = B_LOC // NB  # 4
SW = 32.0  # scale on Wc and W2 (weights ~N(0,0.02) -> ~0.64 in fp8)
SN = 128.0  # scale on N (entries ~0.005 -> ~0.66 in fp8)
WARMUP = 48
FP8 = ml_dtypes.float8_e4m3  # matches mybir.dt.np(float8e4)
DR = mybir.MatmulPerfMode.DoubleRow
AF = mybir.ActivationFunctionType

_DMA_OPCODES = ("DMACopy", "DMATranspose", "EventSemaphore", "TriggeredCopy")


def _legalize_waits(bir: dict) -> dict:
    """Walrus on this stack accepts only ONE sync-wait per engine instruction
    ("Too many sync wait commands"). Hoist extra waits onto standalone
    EventSemaphore ops (what nc.<engine>.wait_ge emits) on the same engine."""
    ctr = 0

    def hoist(out, inst, w):
        nonlocal ctr
        ctr += 1
        out.append(
            {
                "debug": inst.get("debug", 0),
                "engine": inst["engine"],
                "ins": [],
                "outs": [],
                "name": f"I-lgw-{ctr}",
                "opcode": "EventSemaphore",
                "sync_info": {"on_update": [], "on_wait": [w]},
            }
        )

    for fn in bir["functions"]:
        for blk in fn["blocks"]:
            out = []
            for inst in blk["instructions"]:
                si = inst.get("sync_info")
                waits = (si.get("on_wait") or []) if si else []
                op = inst.get("opcode")
                if op == "EventSemaphore":
                    pass
                elif op in ("DMACopy", "DMATranspose", "TriggeredCopy"):
                    # keep one wait (prefer a queue DMA* sem) on the descriptor,
                    # hoist the rest onto the issuing sequencer
                    if len(waits) > 1:
                        keep = [w for w in waits if w["ant_name"].startswith("DMA")]
                        drop = [w for w in waits if not w["ant_name"].startswith("DMA")]
                        if not keep:
                            keep = [waits[-1]]
                            drop = waits[:-1]
                        while len(keep) > 1:
                            drop.append(keep.pop(0))
                        for w in drop:
                            hoist(out, inst, w)
                        si["on_wait"] = keep
                elif len(waits) > 1:
                    for w in waits[:-1]:
                        hoist(out, inst, w)
                    si["on_wait"] = waits[-1:]
                out.append(inst)
            blk["instructions"] = out
    return bir


def _attach_wait_legalizer(nc):
    import json as _json

    orig_fn = nc.to_json_bytes

    def _patched():
        bir = _json.loads(orig_fn())
        _legalize_waits(bir)
        return _json.dumps(bir).encode()

    nc.to_json_bytes = _patched


def build_module(repeat=1):
    nc = bass.Bass()
    f32 = mybir.dt.float32
    fp8 = mybir.dt.float8e4
    bf16 = mybir.dt.bfloat16

    xhT = nc.dram_tensor("xhT", [X, B_LOC], fp8, kind="ExternalInput")
    xlT = nc.dram_tensor("xlT", [X, B_LOC], fp8, kind="ExternalInput")
    whT = nc.dram_tensor("whT", [X, F], fp8, kind="ExternalInput")
    wlT = nc.dram_tensor("wlT", [X, F], fp8, kind="ExternalInput")
    nT = nc.dram_tensor("nT", [X, F], fp8, kind="ExternalInput")
    w2T = nc.dram_tensor("w2T", [F, F], fp8, kind="ExternalInput")
    # biases pre-transposed to SBUF layout [P, 2*KF]: cols 0..KF-1 = gelu
    # bias c, cols KF.. = S+D output bias; single tiny contiguous DMA
    bc = nc.dram_tensor("bc", [P, 2 * KF], f32, kind="ExternalInput")
    outT = nc.dram_tensor("outT", [F, B_LOC], bf16, kind="ExternalOutput")

    xh_ap = xhT[:].rearrange("(k p) b -> p k b", p=P)
    xl_ap = xlT[:].rearrange("(k p) b -> p k b", p=P)
    out_ap = outT[:].rearrange("(k p) b -> p k b", p=P)

    with tile.TileContext(nc) as tc:
        with (
            tc.tile_pool(name="const", bufs=1) as const,
            tc.tile_pool(name="io", bufs=2) as io,
            tc.tile_pool(name="act", bufs=1) as act,
            tc.tile_pool(name="psd", bufs=4, space="PSUM") as psd,
            tc.tile_pool(name="psh", bufs=4, space="PSUM") as psh,
        ):
            blks = [b for _ in range(repeat) for b in range(NBLK)]
            inflight = {}

            def fetch(i):
                bsl = slice(blks[i] * NB, (blks[i] + 1) * NB)
                xh = io.tile([P, KX, NB], fp8, tag="xh")
                nc.sync.dma_start(out=xh, in_=xh_ap[:, :, bsl])
                xl = io.tile([P, KX, NB], fp8, tag="xl")
                nc.sync.dma_start(out=xl, in_=xl_ap[:, :, bsl])
                inflight[i] = (xh, xl)

            # DMA order = need order: H weights + block-0 inputs first so the
            # PE starts ASAP; the S/D weights stream in under the H phase.
            nw = const.tile([P, KX, F], fp8)
            nc.sync.dma_start(out=nw, in_=nT[:].rearrange("(k p) f -> p k f", p=P))
            cv_t = const.tile([P, KF], f32)
            nc.sync.dma_start(out=cv_t, in_=cv[:].rearrange("(k p) -> p k", p=P))
            bsb_t = const.tile([P, KF], f32)
            nc.sync.dma_start(out=bsb_t, in_=bsb[:].rearrange("(k p) -> p k", p=P))
            fetch(0)
            wh = const.tile([P, KX, F], fp8)
            nc.sync.dma_start(out=wh, in_=whT[:].rearrange("(k p) f -> p k f", p=P))
            wl = const.tile([P, KX, F], fp8)
            nc.sync.dma_start(out=wl, in_=wlT[:].rearrange("(k p) f -> p k f", p=P))
            w2 = const.tile([P, KF, F], fp8)
            nc.sync.dma_start(out=w2, in_=w2T[:].rearrange("(k p) f -> p k f", p=P))

            # Warmup: dep-free matmuls on (uninitialized) SBUF keep the PE
            # busy during the initial weight/input DMA and pre-ramp the
            # p-state; results land in a psum tile that is never read.
            warm_w = const.tile([P, 2, P], fp8)
            nc.vector.memset(warm_w, 0.0)
            warm_x = const.tile([P, 2, NB], fp8)
            nc.vector.memset(warm_x, 0.0)
            wps = psh.tile([P, NB], f32, tag="ph")
            for _ in range(WARMUP):
                nc.tensor.matmul(wps, warm_w, warm_x, start=True, stop=True, perf_mode=DR)

            for i, blk in enumerate(blks):
                bsl = slice(blk * NB, (blk + 1) * NB)
                xh, xl = inflight.pop(i)
                if i + 1 < len(blks):
                    fetch(i + 1)

                h = act.tile([P, KF, NB], fp8, tag="h")
                out_t = io.tile([P, KF, NB], bf16, tag="out_t")

                # H: h = gelu(N@xh / SN + c)
                for j in range(KF):
                    ps = psh.tile([P, NB], f32, tag="ph")
                    for k in range(KX // 2):
                        nc.tensor.matmul(
                            ps,
                            nw[:, 2 * k : 2 * k + 2, ts(j, P)],
                            xh[:, 2 * k : 2 * k + 2, :],
                            start=(k == 0),
                            stop=(k == KX // 2 - 1),
                            perf_mode=DR,
                        )
                    nc.scalar.activation(
                        h[:, j, :], ps, AF.Gelu,
                        bias=cv_t[:, j : j + 1], scale=1.0 / SN,
                    )

                # S+D fused per output tile, one accumulation group per bank:
                #   psum = Wh@xh + Wh@xl + Wl@xh + W2@h  (= 32*(gp+tp+W2@h))
                for j in range(KF):
                    ps = psd.tile([P, NB], f32, tag="ps")
                    for w_t, x_t in ((wh, xh), (wh, xl), (wl, xh)):
                        for k in range(KX // 2):
                            nc.tensor.matmul(
                                ps,
                                w_t[:, 2 * k : 2 * k + 2, ts(j, P)],
                                x_t[:, 2 * k : 2 * k + 2, :],
                                start=(w_t is wh and x_t is xh and k == 0),
                                stop=False,
                                perf_mode=DR,
                            )
                    for k in range(KF // 2):
                        nc.tensor.matmul(
                            ps,
                            w2[:, 2 * k : 2 * k + 2, ts(j, P)],
                            h[:, 2 * k : 2 * k + 2, :],
                            start=False,
                            stop=(k == KF // 2 - 1),
                            perf_mode=DR,
                        )
                    nc.scalar.activation(
                        out_t[:, j, :], ps, AF.Identity,
                        bias=bsb_t[:, j : j + 1], scale=1.0 / SW,
                    )
                    if j == KF // 2 - 1:
                        nc.scalar.dma_start(
                            out=out_ap[:, : KF // 2, bsl], in_=out_t[:, : KF // 2, :]
                        )
                nc.scalar.dma_start(
                    out=out_ap[:, KF // 2 :, bsl], in_=out_t[:, KF // 2 :, :]
                )

    _attach_wait_legalizer(nc)
    return nc


def prepare_inputs(gnn_features, transformer_features, Wg, bg, Wt, bt, Wv, bv, Wo, bo, W1, b1, W2, b2):
    """Host-side: fold attention+fusion into N, build hi/lo fp8 splits."""
    f64 = np.float64
    A = Wo.astype(f64) @ Wv.astype(f64)
    W1a = W1[:, :F].astype(f64)
    W1b = W1[:, F:].astype(f64)
    M1 = W1a @ A
    M2 = W1b @ A
    c = (W1a + W1b) @ (Wo.astype(f64) @ bv.astype(f64) + bo.astype(f64)) + b1.astype(f64)
    N1 = M1 @ Wt.astype(f64)  # [F, TR] acts on t
    N2 = M2 @ Wg.astype(f64)  # [F, GNN] acts on g
    c = c + M1 @ bt.astype(f64) + M2 @ bg.astype(f64)

    WcT = np.concatenate([np.asarray(Wg).T, np.asarray(Wt).T], axis=0).astype(np.float32)  # [X, F]
    NcT = np.concatenate([N2.T, N1.T], axis=0).astype(np.float32)  # [X, F]

    wc32 = SW * WcT
    whT = wc32.astype(FP8)
    wlT = (wc32 - whT.astype(np.float32)).astype(FP8)
    nT = (SN * NcT).astype(FP8)
    w2T = (SW * np.ascontiguousarray(np.asarray(W2).T)).astype(FP8)

    shared = {
        "whT": whT,
        "wlT": wlT,
        "nT": nT,
        "w2T": w2T,
        "bsb": (
            np.asarray(bg, np.float32)
            + np.asarray(bt, np.float32)
            + np.asarray(b2, np.float32)
        ),
        "cv": c.astype(np.float32),
    }
    xT_full = np.concatenate(
        [np.asarray(gnn_features), np.asarray(transformer_features)], axis=1
    ).T.astype(np.float32)  # [X, B]
    xh_full = xT_full.astype(FP8)
    xl_full = (xT_full - xh_full.astype(np.float32)).astype(FP8)

    in_maps = []
    for i in range(N_CORES):
        cols = slice(i * B_LOC, (i + 1) * B_LOC)
        in_maps.append(
            {
                "xhT": np.ascontiguousarray(xh_full[:, cols]),
                "xlT": np.ascontiguousarray(xl_full[:, cols]),
                **shared,
            }
        )
    return in_maps


def run(inputs, trace=False, **kw):
    nc = build_module()
    in_maps = prepare_inputs(**inputs)
    res = run_bass_kernel_spmd(nc, in_maps, core_ids=list(range(N_CORES)), trace=trace, **kw)
    out = np.concatenate(
        [r["outT"].astype(np.float32).T for r in res.results], axis=0
    )
    return out, res


def kernel(**inputs) -> np.ndarray:
    out, _ = run(inputs, trace=False)
    return out
